# revision 5
# baseline (speedup 1.0000x reference)
"""Mamba mixer Bass kernel for 8 Trainium2 NeuronCores.

Tensor-parallel over intermediate_size (4096 -> 512 channels per core):
in_proj/conv/scan/gating are fully channel-local; x_proj partials are
AllReduced on-device (160x4096 fp32); out_proj partials are summed on host.

Layouts on device (per core):
  - activations kept as [channel partitions, (b, t) free]
  - hidden_states pre-transposed on host to [H, B*L] so the contraction
    dim lands on partitions for the TensorEngine
  - selective scan runs as one tensor_tensor_scan per (n, d-tile, b):
    state = dA * state + dBu along the time (free) axis
"""

import numpy as np

import concourse.bass as bass
import concourse.mybir as mybir
import concourse.tile as tile
from concourse.bass_utils import run_bass_kernel_spmd

F32 = mybir.dt.float32
F32R = mybir.dt.float32r
AF = mybir.ActivationFunctionType
OP = mybir.AluOpType

N_CORES = 8

CFG_FULL = dict(
    H=2048,      # hidden size
    IL=512,      # local intermediate channels (4096 / 8)
    N=16,        # ssm state size
    R=128,       # dt rank
    B=2,         # batch
    L=2048,      # sequence length
    NB=512,      # bl-chunk for matmul moving dim
    TC=1024,     # time-chunk for the scan working tiles
)


def _split_sync_waits(nc, maxw=1):
    """walrus in this container accepts a single sem-wait per instruction;
    move extra waits onto preceding same-engine drains."""
    cnt = 0
    for bb in nc.main_func.blocks:
        insts = bb.instructions
        i = 0
        while i < len(insts):
            ins = insts[i]
            si = getattr(ins, "sync_info", None)
            waits = list(si.on_wait) if si is not None and si.on_wait else []
            if len(waits) > maxw:
                extra, keep = waits[:-maxw], waits[-maxw:]
                si.on_wait = keep
                pre = []
                for j in range(0, len(extra), maxw):
                    nop = mybir.InstDrain(
                        name=f"{ins.name}-wsplit-{j}", engine=ins.engine)
                    nop.sync_info = mybir.SyncInfo(
                        on_wait=extra[j:j + maxw], on_update=[])
                    pre.append(nop)
                insts[i:i] = pre
                i += len(pre)
                cnt += len(pre)
            i += 1
    return cnt


def build_program(cfg):
    H, IL, N, R, B, L = cfg["H"], cfg["IL"], cfg["N"], cfg["R"], cfg["B"], cfg["L"]
    NB, TC = cfg["NB"], cfg["TC"]
    BL = B * L
    KH = H // 128          # k-tiles of the hidden contraction
    MD = IL // 128         # d-tiles (partition tiles of local channels)
    NBC = BL // NB         # bl chunks
    NTC = L // TC          # time chunks per sequence
    KCONV = 4

    nc = bass.Bass()

    hsT = nc.declare_dram_parameter("hsT", [H, BL], F32, isOutput=False)
    winT = nc.declare_dram_parameter("winT", [H, 2 * IL], F32, isOutput=False)
    convw = nc.declare_dram_parameter("convw", [IL, KCONV], F32, isOutput=False)
    convb = nc.declare_dram_parameter("convb", [IL, 1], F32, isOutput=False)
    xwT = nc.declare_dram_parameter("xwT", [IL, R + 2 * N], F32, isOutput=False)
    dtwT = nc.declare_dram_parameter("dtwT", [R, IL], F32, isOutput=False)
    dtb = nc.declare_dram_parameter("dtb", [IL, 1], F32, isOutput=False)
    Amat = nc.declare_dram_parameter("Amat", [IL, N], F32, isOutput=False)
    Dp = nc.declare_dram_parameter("Dp", [IL, 1], F32, isOutput=False)
    woT = nc.declare_dram_parameter("woT", [IL, H], F32, isOutput=False)
    out_part = nc.declare_dram_parameter("out_part", [H, BL], F32, isOutput=True)

    ssm_part = nc.dram_tensor("ssm_part", [R + 2 * N, BL], F32)
    ssm_full = nc.dram_tensor("ssm_full", [R + 2 * N, BL], F32, addr_space="Shared")
    gate_d = nc.dram_tensor("gate_d", [IL, BL], F32)
    delta_d = nc.dram_tensor("delta_d", [IL, BL], F32)
    du_d = nc.dram_tensor("du_d", [IL, BL], F32)
    y0_d = nc.dram_tensor("y0_d", [IL, BL], F32)
    y_d = nc.dram_tensor("y_d", [IL, BL], F32)

    r32 = lambda ap: ap.bitcast(F32R)

    with tile.TileContext(nc) as tc:
        with tc.tile_pool(name="const", bufs=1) as cp:
            A_t, cw_t, cb_t, db_t, D_t = [], [], [], [], []
            for dt in range(MD):
                rows = slice(dt * 128, (dt + 1) * 128)
                a = cp.tile([128, N], F32, name=f"A{dt}", tag=f"A{dt}")
                nc.sync.dma_start(a[:], Amat[rows, :])
                A_t.append(a)
                w = cp.tile([128, KCONV], F32, name=f"cw{dt}", tag=f"cw{dt}")
                nc.sync.dma_start(w[:], convw[rows, :])
                cw_t.append(w)
                bb_ = cp.tile([128, 1], F32, name=f"cb{dt}", tag=f"cb{dt}")
                nc.sync.dma_start(bb_[:], convb[rows, :])
                cb_t.append(bb_)
                d_ = cp.tile([128, 1], F32, name=f"db{dt}", tag=f"db{dt}")
                nc.sync.dma_start(d_[:], dtb[rows, :])
                db_t.append(d_)
                dd = cp.tile([128, 1], F32, name=f"Dp{dt}", tag=f"Dp{dt}")
                nc.sync.dma_start(dd[:], Dp[rows, :])
                D_t.append(dd)

            # ---- phase A: in_proj + conv + silu; gate spilled to DRAM ----
            with tc.tile_pool(name="u", bufs=1) as up:
                u_t = [up.tile([128, BL], F32R, name=f"u{dt}", tag=f"u{dt}") for dt in range(MD)]
                with tc.tile_pool(name="wA", bufs=1) as wp, \
                     tc.tile_pool(name="hst", bufs=1) as hp, \
                     tc.tile_pool(name="xA", bufs=1) as xp, \
                     tc.tile_pool(name="psA", bufs=4, space="PSUM") as pp, \
                     tc.tile_pool(name="stA", bufs=4) as sp:
                    w_tiles = {}
                    for m in range(2 * MD):
                        for k in range(KH):
                            wt = wp.tile([128, 128], F32R, name=f"w{m}_{k}", tag=f"w{m}_{k}")
                            nc.sync.dma_start(
                                wt[:], winT[k * 128:(k + 1) * 128,
                                            m * 128:(m + 1) * 128].bitcast(F32R))
                            w_tiles[(m, k)] = wt
                    # streaming conv: x chunks are consumed right after
                    # their in_proj matmul; only a 2-chunk window is kept
                    # for the 3-column causal carry
                    NBB = L // NB       # chunks per sequence
                    x_prev = [None] * MD
                    for nb in range(NBC):
                        csl = slice(nb * NB, (nb + 1) * NB)
                        hst = []
                        for k in range(KH):
                            ht = hp.tile([128, NB], F32R, name=f"hst{k}", tag=f"hst{k}")
                            nc.sync.dma_start(
                                ht[:], hsT[k * 128:(k + 1) * 128, csl].bitcast(F32R))
                            hst.append(ht)
                        for m in range(2 * MD):
                            ps = pp.tile([128, NB], F32, name="psA", tag="psA")
                            for k in range(KH):
                                nc.tensor.matmul(
                                    ps[:], w_tiles[(m, k)][:],
                                    hst[k][:],
                                    start=(k == 0), stop=(k == KH - 1))
                            if m < MD:
                                xc = xp.tile([128, NB], F32, name=f"x{m}",
                                             tag=f"x{m}", bufs=2)
                                nc.scalar.copy(xc[:], ps[:])
                                tmp = sp.tile([128, NB], F32, name="ctmp",
                                              tag="ctmp", bufs=3)
                                nc.vector.tensor_scalar_mul(
                                    tmp[:], xc[:], cw_t[m][:, KCONV - 1:KCONV])
                                for s in range(1, KCONV):
                                    nc.vector.scalar_tensor_tensor(
                                        tmp[:, s:], xc[:, :NB - s],
                                        cw_t[m][:, KCONV - 1 - s:KCONV - s],
                                        tmp[:, s:], OP.mult, OP.add)
                                if nb % NBB != 0:
                                    # carry last 3 columns of previous chunk
                                    for s in range(1, KCONV):
                                        nc.vector.scalar_tensor_tensor(
                                            tmp[:, 0:s],
                                            x_prev[m][:, NB - s:NB],
                                            cw_t[m][:, KCONV - 1 - s:KCONV - s],
                                            tmp[:, 0:s], OP.mult, OP.add)
                                nc.scalar.activation(
                                    u_t[m][:, csl], tmp[:], AF.Silu,
                                    bias=cb_t[m][:, 0:1])
                                x_prev[m] = xc
                            else:
                                st = sp.tile([128, NB], F32, name="stA", tag="stA")
                                nc.scalar.activation(st[:], ps[:], AF.Silu)
                                nc.sync.dma_start(
                                    gate_d[(m - MD) * 128:(m - MD + 1) * 128,
                                           csl], st[:])

                # ---- phase B: x_proj partials + AllReduce ----
                with tc.tile_pool(name="xwB", bufs=1) as xwp, \
                     tc.tile_pool(name="psB", bufs=4, space="PSUM") as pbp, \
                     tc.tile_pool(name="stB", bufs=4) as sbp:
                    xw_a, xw_b = [], []
                    for dt in range(MD):
                        rows = slice(dt * 128, (dt + 1) * 128)
                        ta = xwp.tile([128, R], F32R, name=f"xwa{dt}", tag=f"xwa{dt}")
                        nc.sync.dma_start(ta[:], xwT[rows, 0:R].bitcast(F32R))
                        xw_a.append(ta)
                        tb = xwp.tile([128, 2 * N], F32R, name=f"xwb{dt}", tag=f"xwb{dt}")
                        nc.sync.dma_start(tb[:], xwT[rows, R:R + 2 * N].bitcast(F32R))
                        xw_b.append(tb)
                    for nb in range(NBC):
                        csl = slice(nb * NB, (nb + 1) * NB)
                        psa = pbp.tile([R, NB], F32, name="psBa", tag="psBa")
                        psb = pbp.tile([2 * N, NB], F32, name="psBb", tag="psBb")
                        for dt in range(MD):
                            nc.tensor.matmul(psa[:], xw_a[dt][:],
                                             u_t[dt][:, csl],
                                             start=(dt == 0),
                                             stop=(dt == MD - 1))
                        for dt in range(MD):
                            nc.tensor.matmul(psb[:], xw_b[dt][:],
                                             u_t[dt][:, csl],
                                             start=(dt == 0),
                                             stop=(dt == MD - 1))
                        sta = sbp.tile([R, NB], F32, name="stBa", tag="stBa")
                        nc.scalar.copy(sta[:], psa[:])
                        nc.sync.dma_start(ssm_part[0:R, csl], sta[:])
                        stb = sbp.tile([2 * N, NB], F32, name="stBb", tag="stBb")
                        nc.scalar.copy(stb[:], psb[:])
                        nc.sync.dma_start(ssm_part[R:R + 2 * N, csl], stb[:])

                nc.gpsimd.collective_compute(
                    "AllReduce", OP.add,
                    replica_groups=[list(range(N_CORES))],
                    ins=[ssm_part[:, :]],
                    outs=[ssm_full[:, :]],
                )

                # ---- phase C-prep: dt_proj -> delta, du, y0 (= u*D) ----
                with tc.tile_pool(name="dtwC", bufs=1) as dwp, \
                     tc.tile_pool(name="dtlr", bufs=2) as lrp, \
                     tc.tile_pool(name="psC", bufs=2, space="PSUM") as pcp, \
                     tc.tile_pool(name="stC", bufs=8) as scp:
                    dtw_t = []
                    for dt in range(MD):
                        t_ = dwp.tile([R, 128], F32R, name=f"dtw{dt}", tag=f"dtw{dt}")
                        nc.sync.dma_start(
                            t_[:], dtwT[:, dt * 128:(dt + 1) * 128].bitcast(F32R))
                        dtw_t.append(t_)
                    for dt in range(MD):
                        rows = slice(dt * 128, (dt + 1) * 128)
                        for nb in range(NBC):
                            csl = slice(nb * NB, (nb + 1) * NB)
                            lr = lrp.tile([R, NB], F32R, name="dtlr", tag="dtlr")
                            nc.sync.dma_start(lr[:], ssm_full[0:R, csl].bitcast(F32R))
                            ps = pcp.tile([128, NB], F32, name="psC", tag="psC")
                            nc.tensor.matmul(ps[:], dtw_t[dt][:],
                                             lr[:], start=True, stop=True)
                            # softplus(x) = ln(1 + exp(x)); Softplus has no
                            # ACT LUT set in this compiler build
                            ex = scp.tile([128, NB], F32, name="ex", tag="ex")
                            nc.scalar.activation(ex[:], ps[:], AF.Exp,
                                                 bias=db_t[dt][:, 0:1])
                            de = scp.tile([128, NB], F32, name="de", tag="de")
                            nc.scalar.activation(de[:], ex[:], AF.Ln, bias=1.0)
                            nc.sync.dma_start(delta_d[rows, csl], de[:])
                            du = scp.tile([128, NB], F32, name="duC", tag="duC")
                            nc.vector.tensor_mul(du[:], de[:],
                                                 u_t[dt][:, csl].bitcast(F32))
                            nc.sync.dma_start(du_d[rows, csl], du[:])
                            y0 = scp.tile([128, NB], F32, name="y0C", tag="y0C")
                            nc.vector.tensor_scalar_mul(
                                y0[:], u_t[dt][:, csl].bitcast(F32), D_t[dt][:, 0:1])
                            nc.sync.dma_start(y0_d[rows, csl], y0[:])

            # ---- phase C-scan: selective scan + gating ----
            with tc.tile_pool(name="dlS", bufs=1) as dlp, \
                 tc.tile_pool(name="duS", bufs=1) as dup, \
                 tc.tile_pool(name="yaS", bufs=1) as yap, \
                 tc.tile_pool(name="bcS", bufs=4) as bcp, \
                 tc.tile_pool(name="wkS", bufs=2) as wkp, \
                 tc.tile_pool(name="hS", bufs=3) as hsp, \
                 tc.tile_pool(name="gS", bufs=2) as gsp:
                for b in range(B):
                    bh = slice(b * L, (b + 1) * L)
                    dl_b, du_b, ya_b = [], [], []
                    for dt in range(MD):
                        rows = slice(dt * 128, (dt + 1) * 128)
                        d_ = dlp.tile([128, L], F32, name=f"dl{dt}", tag=f"dl{dt}")
                        nc.sync.dma_start(d_[:], delta_d[rows, bh])
                        dl_b.append(d_)
                        m_ = dup.tile([128, L], F32, name=f"duS{dt}", tag=f"duS{dt}")
                        nc.sync.dma_start(m_[:], du_d[rows, bh])
                        du_b.append(m_)
                        y_ = yap.tile([128, L], F32, name=f"ya{dt}", tag=f"ya{dt}")
                        nc.sync.dma_start(y_[:], y0_d[rows, bh])
                        ya_b.append(y_)
                    for n in range(N):
                        bbc, cbc = [], []
                        for ti in range(NTC):
                            tsl = slice(b * L + ti * TC, b * L + (ti + 1) * TC)
                            bt = bcp.tile([128, TC], F32, name="Bbc", tag="Bbc")
                            nc.sync.dma_start(
                                bt[:],
                                ssm_full[R + n:R + n + 1,
                                         tsl].to_broadcast((128, TC)))
                            bbc.append(bt)
                            ct = bcp.tile([128, TC], F32, name="Cbc", tag="Cbc")
                            nc.sync.dma_start(
                                ct[:],
                                ssm_full[R + N + n:R + N + n + 1,
                                         tsl].to_broadcast((128, TC)))
                            cbc.append(ct)
                        for dt in range(MD):
                            hprev = None
                            for ti in range(NTC):
                                tsl = slice(ti * TC, (ti + 1) * TC)
                                dA = wkp.tile([128, TC], F32, name="dA", tag="dA")
                                nc.scalar.activation(
                                    dA[:], dl_b[dt][:, tsl], AF.Exp,
                                    scale=A_t[dt][:, n:n + 1])
                                dBu = wkp.tile([128, TC], F32, name="dBu", tag="dBu")
                                nc.vector.tensor_mul(
                                    dBu[:], du_b[dt][:, tsl], bbc[ti][:])
                                h = hsp.tile([128, TC], F32, name="h", tag="h")
                                init = 0.0 if ti == 0 else hprev[:, TC - 1:TC]
                                nc.vector.tensor_tensor_scan(
                                    h[:], dA[:], dBu[:], init,
                                    op0=OP.mult, op1=OP.add)
                                hc = wkp.tile([128, TC], F32, name="hc", tag="hc")
                                nc.vector.tensor_mul(hc[:], h[:], cbc[ti][:])
                                nc.vector.tensor_add(
                                    ya_b[dt][:, tsl], ya_b[dt][:, tsl], hc[:])
                                hprev = h
                    for dt in range(MD):
                        rows = slice(dt * 128, (dt + 1) * 128)
                        g = gsp.tile([128, L], F32, name="g", tag="g")
                        nc.sync.dma_start(g[:], gate_d[rows, bh])
                        yg = gsp.tile([128, L], F32, name="yg", tag="yg")
                        nc.vector.tensor_mul(yg[:], ya_b[dt][:], g[:])
                        nc.sync.dma_start(y_d[rows, bh], yg[:])

            # ---- phase D: out_proj partials ----
            with tc.tile_pool(name="woD", bufs=1) as wop, \
                 tc.tile_pool(name="yD", bufs=2) as ydp, \
                 tc.tile_pool(name="psD", bufs=4, space="PSUM") as pdp, \
                 tc.tile_pool(name="stD", bufs=4) as sdp:
                wo_t = {}
                for kk in range(MD):
                    for m in range(KH):
                        wt = wop.tile([128, 128], F32R, name=f"wo{kk}_{m}", tag=f"wo{kk}_{m}")
                        nc.sync.dma_start(
                            wt[:], woT[kk * 128:(kk + 1) * 128,
                                       m * 128:(m + 1) * 128].bitcast(F32R))
                        wo_t[(kk, m)] = wt
                for nb in range(NBC):
                    csl = slice(nb * NB, (nb + 1) * NB)
                    yts = []
                    for kk in range(MD):
                        yt = ydp.tile([128, NB], F32R, name=f"yD{kk}", tag=f"yD{kk}")
                        nc.sync.dma_start(
                            yt[:], y_d[kk * 128:(kk + 1) * 128, csl].bitcast(F32R))
                        yts.append(yt)
                    for m in range(KH):
                        ps = pdp.tile([128, NB], F32, name="psD", tag="psD")
                        for kk in range(MD):
                            nc.tensor.matmul(ps[:], wo_t[(kk, m)][:],
                                             yts[kk][:],
                                             start=(kk == 0),
                                             stop=(kk == MD - 1))
                        st = sdp.tile([128, NB], F32, name="stD", tag="stD")
                        nc.scalar.copy(st[:], ps[:])
                        nc.sync.dma_start(
                            out_part[m * 128:(m + 1) * 128, csl], st[:])

    _split_sync_waits(nc)
    return nc


def make_in_maps(cfg, hidden_states, in_proj_w, conv_w, conv_b, x_proj_w,
                 dt_proj_w, dt_proj_b, A_log, D_param, out_proj_w):
    H, IL, N, R, B, L = cfg["H"], cfg["IL"], cfg["N"], cfg["R"], cfg["B"], cfg["L"]
    BL = B * L
    I_full = IL * N_CORES
    c = np.ascontiguousarray
    hsT = c(hidden_states.reshape(BL, H).T.astype(np.float32))
    A_full = -np.exp(A_log.astype(np.float32))
    in_maps = []
    for ci in range(N_CORES):
        sl = slice(ci * IL, (ci + 1) * IL)
        gsl = slice(I_full + ci * IL, I_full + (ci + 1) * IL)
        wxT = in_proj_w[sl, :].T
        wgT = in_proj_w[gsl, :].T
        in_maps.append({
            "hsT": hsT,
            "winT": c(np.concatenate([wxT, wgT], axis=1).astype(np.float32)),
            "convw": c(conv_w[sl, 0, :].astype(np.float32)),
            "convb": c(conv_b[sl].reshape(IL, 1).astype(np.float32)),
            "xwT": c(x_proj_w[:, sl].T.astype(np.float32)),
            "dtwT": c(dt_proj_w[sl, :].T.astype(np.float32)),
            "dtb": c(dt_proj_b[sl].reshape(IL, 1).astype(np.float32)),
            "Amat": c(A_full[sl, :]),
            "Dp": c(D_param[sl].reshape(IL, 1).astype(np.float32)),
            "woT": c(out_proj_w[:, sl].T.astype(np.float32)),
        })
    return in_maps


_PROG_CACHE = {}


def run(cfg, inputs, **run_kwargs):
    key = tuple(sorted(cfg.items()))
    if key not in _PROG_CACHE:
        _PROG_CACHE[key] = build_program(cfg)
    nc = _PROG_CACHE[key]
    in_maps = make_in_maps(cfg, **inputs)
    res = run_bass_kernel_spmd(nc, in_maps, list(range(N_CORES)), **run_kwargs)
    H, B, L = cfg["H"], cfg["B"], cfg["L"]
    out = np.zeros((H, B * L), np.float64)
    for ci in range(N_CORES):
        out += res.results[ci]["out_part"]
    full = out.astype(np.float32).T.reshape(B, L, H)
    return full, res


def kernel(**inputs):
    out, _ = run(CFG_FULL, inputs)
    return out
